# revision 14
# baseline (speedup 1.0000x reference)
"""Trainium2 Bass kernel for masked cross-attention (sparse_attention).

Reference computation (per batch b):
    q = x @ Wq + bq                      # [N, hd]   (hd = 8 heads * 32)
    k = ctx @ Wk + bk ; v = ctx @ Wv + bv
    dots[h,i,j] = q_h[i] . k_h[j]  + frag_mask[j]   (masked j -> -inf)
    attn = softmax_j(dots) ; out = (attn @ v) @ W_out + b_out

Distribution: 8 cores = 4 batches x 2 head-groups (4 heads each).
Host-side prep: compact context along j by the boolean mask (~50% kept),
transpose x/context to [dim, tokens] layout, slice weights per head group.

Key structure (v2):
  - Wk is pre-scaled by log2(e) so the S matmul produces x = log2e*s
    directly.  bk is dropped exactly (a per-(h,i) constant shift cancels in
    softmax).  exp(frag_mask) is folded multiplicatively into the V rows and
    the L (denominator) stationary, with 0.0 exactly killing padded j.
  - Phase 1: K/V projections for all j-tiles, pipelined with the DMA loads.
    K psum staging in spair banks 0/2 (ping-pong), V in banks 1/3; K evicts
    on ACT (copy->fp16), V evicts on DVE (scale by exp(frag)->bf16).
  - Phase 2 per j-tile: S matmuls row-tiled per head (tile_position=(32h,0),
    concurrent across heads) write a 4-bank pair-region 'spair' with
    bank-aligned zones: [h0_act|h1_act|h0_dve|h1_dve] x 512 i-columns.
    The softmax exponentials are split across TWO engines:
      * ACT: activation(Exp, scale=ln2) on the first 1024 columns
      * DVE: custom single-pass EXPQ op (magic-add round + raw-bit shift
        exponent construction + (1+br)^2 mantissa poly; see expq build
        below) on the last 1024 columns.  Global scale factors cancel in
        the softmax normalize (consistent per i-column).
    PV and the denominator L accumulate via col-tiled (tile_position=
    (0,32h)) bf16 matmuls, concurrent across heads, start=False into
    memset accumulators (no whole-bank has_written clears racing
    concurrent siblings).
  - normalize with DVE reciprocal_approx_fast + mult; project with W_out
    (v-bias and b_out folded into the host-side output assembly, exactly).
"""

import numpy as np
import ml_dtypes

import concourse.bass as bass  # noqa: F401
import concourse.mybir as mybir
import concourse.tile as tile
import concourse.bacc as bacc
from concourse.bass_utils import run_bass_kernel_spmd

import concourse.dve_ops as dve_ops
from concourse.dve_ops import DveOp
from concourse.dve_spec import Spec, Src0
from concourse.dve_uop import (
    AluInp, AluOp, DelayInp, DveOpSpec, InpSel, OutPath, OutSel, Trigger,
    UopConfig, ENABLE,
)

F32 = mybir.dt.float32
F16 = mybir.dt.float16
BF16 = mybir.dt.bfloat16
AF = mybir.ActivationFunctionType

B = 4
N_Q = 1024          # queries per batch
DIM = 256           # model dim
D_HEAD = 32
HPC = 4             # heads per core
HD = 128            # HPC * D_HEAD: head-group width
LOG2E = 1.4426950408889634
LN2 = 0.6931471805599453

# ---------------------------------------------------------------------------
# EXPQ_ANT: single-pass custom-DVE exponential.
#   in:  x = log2e * s  (fp32, any AP)
#   out: K * 2^x  with a global constant K (cancels in softmax)
# Chain: z = x+MAGIC (round to int a); d = z-MAGIC; r = x-d in [-.5,.5];
#   e1 = bits((z & 0xff) << 23) = 2^(a-63)  [byte = a+64, needs |x| < 63]
#   out = (1 + BETA*r)^2 * e1      max rel err 1.5e-2 about global scale
# The <<23 shift count rides CONST_2 as an fp32 denormal with raw bits 23
# (verified bit-exact on HW).
# ---------------------------------------------------------------------------
EXPQ_MAGIC = 12582912.0 + 64.0
EXPQ_BETA = 0.34314575
EXPQ_SHIFT23 = float(np.uint32(23).view(np.float32))


def _expq_ref(x):
    x = np.asarray(x, dtype=np.float32)
    z = (x + np.float32(EXPQ_MAGIC)).astype(np.float32)
    d = (z - np.float32(EXPQ_MAGIC)).astype(np.float32)
    r = (x - d).astype(np.float32)
    e1 = ((z.view(np.uint32) & 0xFF).astype(np.uint32) << 23).view(np.float32)
    t2 = (1.0 + np.float32(EXPQ_BETA) * r).astype(np.float32)
    return (t2 * t2 * e1).astype(np.float32)


def _expq_uop() -> UopConfig:
    u = UopConfig()
    u.enable_input(InpSel.SRC_0, 1)     # d0: x
    u.enable_input(InpSel.CONST_0, 2)   # d1: MAGIC
    u.enable_input(InpSel.CONST_1, 3)   # d2: BETA
    u.enable_input(InpSel.CONST_2, 4)   # d3: shift count (raw 23)
    u.enable_input(InpSel.ONE_F32, 5)   # d4: 1.0
    u.require_inp0 = ENABLE
    u.trigger = (Trigger.SRC_TENSOR_DONE, Trigger.NONE, Trigger.NONE)
    D = u.datapath_config
    D[0].enable_alu(AluOp.ADD, AluInp.PREV_DELAY_0, AluInp.PREV_DELAY_1)
    D[0].pass_through_delay(0, 1, 2, 3, 4)
    D[1].enable_alu(AluOp.SUBTRACT, AluInp.PREV_ALU_OUT, AluInp.PREV_DELAY_1)
    D[1].pass_through_delay(0, 2, 3, 4)
    D[1].enable_delay_from_src(DelayInp.PREV_ALU_OUT, 1)  # z
    D[2].enable_alu(AluOp.SUBTRACT, AluInp.PREV_DELAY_0, AluInp.PREV_ALU_OUT)
    D[2].pass_through_delay(1, 2, 3, 4)
    D[3].enable_alu(
        AluOp.LOGICAL_SHIFT_LEFT, AluInp.PREV_DELAY_1, AluInp.PREV_DELAY_3)
    D[3].pass_through_delay(2, 4)
    D[3].enable_delay_from_src(DelayInp.PREV_ALU_OUT, 0)  # r
    D[4].enable_alu(AluOp.MULTIPLY, AluInp.PREV_DELAY_0, AluInp.PREV_DELAY_2)
    D[4].pass_through_delay(4)
    D[4].enable_delay_from_src(DelayInp.PREV_ALU_OUT, 1)  # e1
    D[5].enable_alu(AluOp.ADD, AluInp.PREV_ALU_OUT, AluInp.PREV_DELAY_4)
    D[5].pass_through_delay(1)
    D[6].enable_alu(AluOp.MULTIPLY, AluInp.PREV_ALU_OUT, AluInp.PREV_ALU_OUT)
    D[6].pass_through_delay(1)
    D[7].enable_alu(AluOp.MULTIPLY, AluInp.PREV_ALU_OUT, AluInp.PREV_DELAY_1)
    u.enable_output(OutSel.ALU_OUT, OutPath.WR0_LO)
    return u


class _ExpqOp(DveOp):
    def __init__(self):
        object.__setattr__(self, "name", "EXPQ_ANT")
        object.__setattr__(
            self, "spec", Spec(body=Src0, reference=lambda *a: _expq_ref(a[0])))
        object.__setattr__(self, "subdim", False)
        object.__setattr__(self, "uops_sha", {})
        object.__setattr__(self, "perf_en", {})

    def compile(self, ver):
        return DveOpSpec(
            name=self.name,
            opcode=dve_ops.get_dve_sub_opcode(self.name),
            uops=[_expq_uop()],
            rd1_en=False,
        )


def _expq_register():
    if "EXPQ_ANT" in dve_ops._SUB_OPCODE_FOR_NAME:
        return next(op for op in dve_ops.OPS if op.name == "EXPQ_ANT")
    op = _ExpqOp()
    dve_ops.OPS.append(op)
    row = dve_ops._CUSTOM_DVE_ROW_BASE + len(dve_ops.OPS) - 1
    assert row < 0x20, "custom-DVE row overflow"
    dve_ops._SUB_OPCODE_FOR_NAME[op.name] = row
    dve_ops.CUSTOM_DVE_SPECS[op.name] = op.spec
    return op


def _expq_emit(nc, out_ap, in_ap, bias=None):
    op = _expq_register()
    return nc.vector._custom_dve(
        op, out=out_ap, in0=in_ap,
        s0=EXPQ_MAGIC if bias is None else bias,
        s1=EXPQ_BETA, imm2=EXPQ_SHIFT23)


_cache: dict = {}
last_results = None  # test.py introspection


def _build(mjt: int, reps: int = 1, debug: bool = False, exp_mode: str = 'dve', do_pvl: bool = True, do_phase1: bool = True, pvl_mode: str = 'both', ebufs: int = 14):
    """Build + compile the per-core Bass program for mjt j-tiles of 128."""
    mp = mjt * 128
    nc = bacc.Bacc("TRN2", target_bir_lowering=False, debug=False)

    d_xT = nc.declare_dram_parameter("xT", [2, 128, N_Q], F16, isOutput=False)
    d_cT = nc.declare_dram_parameter("cT", [2, 128, mp], F16, isOutput=False)
    d_wqk = nc.declare_dram_parameter("wqk", [128, 4 * HD], F16, isOutput=False)
    d_wv = nc.declare_dram_parameter("wv", [128, 2 * HD], F16, isOutput=False)
    d_wo = nc.declare_dram_parameter("wo", [128, DIM], F16, isOutput=False)
    d_bq = nc.declare_dram_parameter("bq", [128, 1], F32, isOutput=False)
    d_ef = nc.declare_dram_parameter("expf", [128, mjt], F32, isOutput=False)
    d_efw = nc.declare_dram_parameter(
        "expfw", [128, 32 * mjt], BF16, isOutput=False)
    d_out = nc.declare_dram_parameter("outT", [2, 128, N_Q], F16, isOutput=True)
    if debug:
        d_dbg = {
            nm: nc.declare_dram_parameter(f"dbg_{nm}", [128, width], F32,
                                          isOutput=True)
            for nm, width in [("l", N_Q), ("pv", N_Q), ("q", N_Q), ("k", mp),
                              ("v", mp), ("at", N_Q)]
        }

    with tile.TileContext(nc) as tc:
        with (
            tc.tile_pool(name="pin", bufs=1) as pin,
            tc.tile_pool(name="pwork", bufs=1) as pwork,
            tc.tile_pool(name="pe", bufs=ebufs) as pe_pool,
            tc.tile_pool(name="ps_s", bufs=4, space="PSUM") as ps_s,
            tc.tile_pool(name="ps_acc", bufs=1, space="PSUM") as ps_acc,
        ):
          for _rep in range(reps):
            # ---- loads ----
            xT_f = pin.tile([128, 2 * N_Q], F16)
            wqk_f = pin.tile([128, 4 * HD], F16)
            wv_f = pin.tile([128, 2 * HD], F16)
            bq_t = pin.tile([128, 1], F32)
            wq = [wqk_f[:, i * HD:(i + 1) * HD] for i in range(2)]
            wk = [wqk_f[:, (2 + i) * HD:(3 + i) * HD] for i in range(2)]
            wv = [wv_f[:, i * HD:(i + 1) * HD] for i in range(2)]
            cT_f = pin.tile([128, 2 * mp], F16)
            cT = [cT_f[:, i * mp:(i + 1) * mp] for i in range(2)]
            mh = (mp + 1) // 2
            ef = pin.tile([128, mjt], F32)
            efw = pin.tile([128, 32 * mjt], BF16)
            nc.sync.dma_start(wqk_f[:], d_wqk[:])
            nc.sync.dma_start(xT_f[:, 0:N_Q], d_xT[0])
            for ct in range(2):
                nc.sync.dma_start(cT_f[:, ct * mp:ct * mp + 256],
                                  d_cT[ct][:, 0:256])
            nc.sync.dma_start(bq_t[:], d_bq[:])
            nc.sync.dma_start(ef[:], d_ef[:])
            for ct in range(2):
                nc.sync.dma_start(
                    cT_f[:, ct * mp + 256:ct * mp + mh], d_cT[ct][:, 256:mh])
            nc.gpsimd.dma_start(xT_f[:, N_Q:2 * N_Q], d_xT[1])
            nc.gpsimd.dma_start(wv_f[:], d_wv[:])
            wo = pin.tile([128, DIM], F16)
            nc.gpsimd.dma_start(wo[:], d_wo[:])
            nc.gpsimd.dma_start(efw[:], d_efw[:])
            for ct in range(2):
                nc.gpsimd.dma_start(cT_f[:, ct * mp + mh:(ct + 1) * mp],
                                    d_cT[ct][:, mh:mp])

            # ---- persistent SBUF working tensors ----
            qT_hi = pwork.tile([128, N_Q], F16)   # [head*dim, i]
            kT_hi = pwork.tile([128, mp], F16)    # [head*dim, j] (log2e-scaled)
            vnat = pwork.tile([128, mp], BF16)    # [j_local, jt*128+head*dim]
            attnT = pwork.tile([128, N_Q], F16)
            linv = pwork.tile([128, N_Q], F32)
            outT = [pwork.tile([128, N_Q], F16, tag=f"outT{i}", name=f"outT{i}")
                    for i in range(2)]

            ones32 = pin.tile([128, 32], BF16)
            nc.vector.memset(ones32[:], 1.0)

            # warm the ACT exp table during the DMA phase
            warm = pwork.tile([128, 1], F32, tag="warm")
            nc.vector.memset(warm[:], 0.0)
            warm2 = pwork.tile([128, 1], F32, tag="warm2")
            nc.scalar.activation(warm2[:], warm[:], AF.Exp)

            # ---- PSUM accumulators ----
            pv_acc = ps_acc.tile([128, N_Q], F32, tag="pv")
            l_acc = ps_acc.tile([128, N_Q], F32, tag="l")
            nc.vector.memset(pv_acc[:], 0.0)
            nc.vector.memset(l_acc[:], 0.0)

            # ---- Q^T projection via pool tiles, evict with +bq ----
            for ih in range(2):
                sl = slice(ih * 512, ih * 512 + 512)
                qps = ps_s.tile([128, 512], F32, tag="s")
                for ct in range(2):
                    x0 = ih * N_Q + ct * 512
                    nc.tensor.matmul(
                        qps[:], wq[ct][:], xT_f[:, x0:x0 + 512],
                        start=(ct == 0), stop=(ct == 1),
                    )
                nc.vector.tensor_scalar_add(qT_hi[:, sl], qps[:], bq_t[:])

            # ---- phase 1: K/V projections for all j-tiles ----
            # K stages in spair banks 0/2 (jt parity), V in banks 1/3.
            def emit_kv(jt):
                j0 = jt * 128
                kps = ps_s.tile([128, 512], F32, tag="s", name="kps")
                for ct in range(2):
                    nc.tensor.matmul(
                        kps[:, 0:128], wk[ct][:], cT[ct][:, j0:j0 + 128],
                        start=(ct == 0), stop=(ct == 1),
                    )
                nc.scalar.copy(kT_hi[:, j0:j0 + 128], kps[:, 0:128])
                vps = ps_s.tile([128, 512], F32, tag="s", name="vps")
                for ct in range(2):
                    nc.tensor.matmul(
                        vps[:, 0:128], cT[ct][:, j0:j0 + 128], wv[ct][:],
                        start=(ct == 0), stop=(ct == 1),
                    )
                nc.vector.tensor_scalar_mul(
                    vnat[:, j0:j0 + 128], vps[:, 0:128], ef[:, jt:jt + 1])

            for jt in range(min(2, mjt)):
                emit_kv(jt)

            # ---- phase 2: half-rounds (head, i-half), depth-4 pipeline ----
            # S regions: 4 x [128,512] psum banks, cycled by round index.
            # Each round: 1 S matmul -> exp([128,512]) -> e tile; PV/L of the
            # previous tile's matching round interleave on the PE.
            def emit_pvl_hc(jt_p, h, c, e_t, last):
                j0p = jt_p * 128
                isl = slice(c * 512, c * 512 + 512)
                fin = last
                nc.tensor.matmul(
                    pv_acc[32 * h:32 * h + 32, isl],
                    vnat[:, j0p + 32 * h:j0p + 32 * h + 32], e_t[:],
                    start=False, stop=fin,
                    tile_position=(0, 32 * h),
                    skip_group_check=True,
                )
                nc.tensor.matmul(
                    l_acc[32 * h:32 * h + 32, isl],
                    efw[:, 32 * jt_p:32 * jt_p + 32], e_t[:],
                    start=False, stop=fin,
                    tile_position=(0, 32 * h),
                    skip_group_check=True,
                )

            prev = None  # (jt, {(h,c): e_tile})
            for jt in range(mjt):
                j0 = jt * 128
                e_map = {}
                r = 0
                for h in range(HPC):
                    hp = slice(32 * h, 32 * h + 32)
                    for c in range(2):
                        reg_t = ps_s.tile([128, 512], F32, tag="s", name="sreg")
                        reg = reg_t[:]
                        nc.tensor.matmul(
                            reg, kT_hi[hp, j0:j0 + 128],
                            qT_hi[hp, c * 512:c * 512 + 512],
                            start=True, stop=True,
                            tile_position=(32 * h, 0),
                        )
                        e_t = pe_pool.tile([128, 512], BF16, tag="e")
                        use_dve = (exp_mode == "dve") or (
                            exp_mode == "split" and c == 1)
                        if use_dve:
                            _expq_emit(nc, e_t[:], reg)
                        else:
                            nc.scalar.activation(e_t[:], reg, AF.Exp,
                                                 scale=LN2)
                        e_map[(h, c)] = e_t
                        if prev is not None:
                            emit_pvl_hc(prev[0], h, c, prev[1][(h, c)],
                                        last=False)
                        r += 1
                        if r == 4 and jt + 2 < mjt:
                            emit_kv(jt + 2)
                prev = (jt, e_map)
                if jt == mjt - 1:
                    for h in range(HPC):
                        for c in range(2):
                            emit_pvl_hc(prev[0], h, c, prev[1][(h, c)],
                                        last=(h == HPC - 1 and c == 1))
                    prev = None

            # ---- debug dumps ----
            if debug:
                for nm, src, w in [("l", l_acc, N_Q), ("pv", pv_acc, N_Q),
                                   ("q", qT_hi, N_Q), ("k", kT_hi, mp),
                                   ("v", vnat, mp)]:
                    dt = pwork.tile([128, w], F32, tag=f"dbg_{nm}")
                    nc.vector.tensor_copy(dt[:], src[:])
                    nc.sync.dma_start(d_dbg[nm][:], dt[:])

            # ---- normalize + output projection ----
            for ih in range(2):
                sl = slice(ih * 512, ih * 512 + 512)
                nc.vector.reciprocal_approx_fast(linv[:, sl], l_acc[:, sl])
                nc.vector.tensor_tensor(
                    attnT[:, sl], pv_acc[:, sl], linv[:, sl],
                    mybir.AluOpType.mult)
                for dt in range(2):
                    ops_ = ps_s.tile([128, 512], F32, tag="s")
                    nc.tensor.matmul(
                        ops_[:], wo[:, dt * 128:dt * 128 + 128],
                        attnT[:, sl],
                        start=True, stop=True,
                    )
                    nc.scalar.copy(outT[dt][:, sl], ops_[:])
                    nc.sync.dma_start(d_out[dt][:, sl], outT[dt][:, sl])
            if debug:
                dt2 = pwork.tile([128, N_Q], F32, tag="dbg_at")
                nc.vector.tensor_copy(dt2[:], attnT[:])
                nc.sync.dma_start(d_dbg["at"][:], dt2[:])

    nc.compile()
    return nc


def build_in_maps(inputs, keeps, mjt):
    x = np.ascontiguousarray(np.asarray(inputs["x"], dtype=np.float32))
    context = np.ascontiguousarray(
        np.asarray(inputs["context"], dtype=np.float32))
    frag_mask = np.asarray(inputs["frag_mask"], dtype=np.float32)
    W_qkv = np.ascontiguousarray(np.asarray(inputs["W_qkv"], dtype=np.float32))
    b_qkv = np.asarray(inputs["b_qkv"], dtype=np.float32)
    W_out = np.ascontiguousarray(np.asarray(inputs["W_out"], dtype=np.float32))
    mp = mjt * 128
    in_maps = []
    for core in range(8):
        b, hh = core % B, core // B
        keep = keeps[b]
        cnt = len(keep)
        cT = np.zeros((DIM, mp), dtype=np.float32)
        cT[:, :cnt] = context[b][keep].T
        expf = np.zeros((mp,), dtype=np.float32)
        expf[:cnt] = np.exp(frag_mask[b][keep])
        hs = slice(hh * HD, (hh + 1) * HD)
        wq2 = W_qkv[:, hs].reshape(2, 128, HD)
        wk2 = (W_qkv[:, 256:512][:, hs] * np.float32(LOG2E)).reshape(2, 128, HD)
        wv2 = W_qkv[:, 512:768][:, hs].reshape(2, 128, HD)
        xr = x[b].T.reshape(2, 128, N_Q)
        efw = np.repeat(expf.reshape(mjt, 128).T[:, :, None], 32,
                        axis=2).reshape(128, mjt * 32)
        xih = np.stack([
            np.concatenate([xr[0][:, 0:512], xr[1][:, 0:512]], axis=1),
            np.concatenate([xr[0][:, 512:1024], xr[1][:, 512:1024]], axis=1),
        ])
        in_maps.append({
            "xT": np.ascontiguousarray(xih).astype(np.float16),
            "cT": np.ascontiguousarray(cT.reshape(2, 128, mp)).astype(
                np.float16),
            "wqk": np.ascontiguousarray(
                np.concatenate([wq2[0], wq2[1], wk2[0], wk2[1]], axis=1)
            ).astype(np.float16),
            "wv": np.ascontiguousarray(
                np.concatenate([wv2[0], wv2[1]], axis=1)).astype(np.float16),
            "wo": np.ascontiguousarray(W_out[hs, :]).astype(np.float16),
            "bq": np.ascontiguousarray(b_qkv[0:256][hs].reshape(128, 1)),
            "expf": np.ascontiguousarray(expf.reshape(mjt, 128).T),
            "expfw": np.ascontiguousarray(efw).astype(ml_dtypes.bfloat16),
        })
    return in_maps


def kernel(x, context, mask, frag_mask, W_qkv, b_qkv, W_out, b_out):
    global last_results
    mask = np.asarray(mask).astype(bool)
    b_out = np.asarray(b_out, dtype=np.float32)

    keeps = [np.nonzero(mask[b])[0] for b in range(B)]
    mjt = max(1, max((len(k) + 127) // 128 for k in keeps))

    key = (mjt, 1, False)
    if key not in _cache:
        _cache[key] = _build(mjt)
    nc = _cache[key]

    inputs = {"x": x, "context": context, "frag_mask": frag_mask,
              "W_qkv": W_qkv, "b_qkv": b_qkv, "W_out": W_out}
    in_maps = build_in_maps(inputs, keeps, mjt)

    res = run_bass_kernel_spmd(nc, in_maps, list(range(8)))
    last_results = res

    out = np.zeros((B, N_Q, DIM), dtype=np.float32)
    for core in range(8):
        b = core % B
        partial = res.results[core]["outT"].astype(np.float32).reshape(DIM, N_Q)
        out[b] += partial.T
    b_qkv = np.asarray(b_qkv, dtype=np.float32)
    out += (b_out + b_qkv[512:768] @ np.asarray(W_out, dtype=np.float32))[
        None, None, :]
    return out


# revision 15
# speedup vs baseline: 1.3204x; 1.3204x over previous
"""Trainium2 Bass kernel for masked cross-attention (sparse_attention).

Reference computation (per batch b):
    q = x @ Wq + bq                      # [N, hd]   (hd = 8 heads * 32)
    k = ctx @ Wk + bk ; v = ctx @ Wv + bv
    dots[h,i,j] = q_h[i] . k_h[j]  + frag_mask[j]   (masked j -> -inf)
    attn = softmax_j(dots) ; out = (attn @ v) @ W_out + b_out

Distribution: 8 cores = 4 batches x 2 head-groups (4 heads each).
Host-side prep: compact context along j by the boolean mask (~50% kept),
transpose x/context to [dim, tokens] layout, slice weights per head group.

Key structure (v2):
  - Wk is pre-scaled by log2(e) so the S matmul produces x = log2e*s
    directly.  bk is dropped exactly (a per-(h,i) constant shift cancels in
    softmax).  exp(frag_mask) is folded multiplicatively into the V rows and
    the L (denominator) stationary, with 0.0 exactly killing padded j.
  - Phase 1: K/V projections for all j-tiles, pipelined with the DMA loads.
    K psum staging in spair banks 0/2 (ping-pong), V in banks 1/3; K evicts
    on ACT (copy->fp16), V evicts on DVE (scale by exp(frag)->bf16).
  - Phase 2 per j-tile: S matmuls row-tiled per head (tile_position=(32h,0),
    concurrent across heads) write a 4-bank pair-region 'spair' with
    bank-aligned zones: [h0_act|h1_act|h0_dve|h1_dve] x 512 i-columns.
    The softmax exponentials are split across TWO engines:
      * ACT: activation(Exp, scale=ln2) on the first 1024 columns
      * DVE: custom single-pass EXPQ op (magic-add round + raw-bit shift
        exponent construction + (1+br)^2 mantissa poly; see expq build
        below) on the last 1024 columns.  Global scale factors cancel in
        the softmax normalize (consistent per i-column).
    PV and the denominator L accumulate via col-tiled (tile_position=
    (0,32h)) bf16 matmuls, concurrent across heads, start=False into
    memset accumulators (no whole-bank has_written clears racing
    concurrent siblings).
  - normalize with DVE reciprocal_approx_fast + mult; project with W_out
    (v-bias and b_out folded into the host-side output assembly, exactly).
"""

import numpy as np
import ml_dtypes

import concourse.bass as bass  # noqa: F401
import concourse.mybir as mybir
import concourse.tile as tile
import concourse.bacc as bacc
from concourse.bass_utils import run_bass_kernel_spmd

import concourse.dve_ops as dve_ops
from concourse.dve_ops import DveOp
from concourse.dve_spec import Spec, Src0
from concourse.dve_uop import (
    AluInp, AluOp, DelayInp, DveOpSpec, InpSel, OutPath, OutSel, Trigger,
    UopConfig, ENABLE,
)

F32 = mybir.dt.float32
F16 = mybir.dt.float16
BF16 = mybir.dt.bfloat16
AF = mybir.ActivationFunctionType

B = 4
N_Q = 1024          # queries per batch
DIM = 256           # model dim
D_HEAD = 32
HPC = 4             # heads per core
HD = 128            # HPC * D_HEAD: head-group width
LOG2E = 1.4426950408889634
LN2 = 0.6931471805599453

# ---------------------------------------------------------------------------
# EXPQ_ANT: single-pass custom-DVE exponential.
#   in:  x = log2e * s  (fp32, any AP)
#   out: K * 2^x  with a global constant K (cancels in softmax)
# Chain: z = x+MAGIC (round to int a); d = z-MAGIC; r = x-d in [-.5,.5];
#   e1 = bits((z & 0xff) << 23) = 2^(a-63)  [byte = a+64, needs |x| < 63]
#   out = (1 + BETA*r)^2 * e1      max rel err 1.5e-2 about global scale
# The <<23 shift count rides CONST_2 as an fp32 denormal with raw bits 23
# (verified bit-exact on HW).
# ---------------------------------------------------------------------------
EXPQ_MAGIC = 12582912.0 + 64.0
EXPQ_BETA = 0.34314575
EXPQ_SHIFT23 = float(np.uint32(23).view(np.float32))


def _expq_ref(x):
    x = np.asarray(x, dtype=np.float32)
    z = (x + np.float32(EXPQ_MAGIC)).astype(np.float32)
    d = (z - np.float32(EXPQ_MAGIC)).astype(np.float32)
    r = (x - d).astype(np.float32)
    e1 = ((z.view(np.uint32) & 0xFF).astype(np.uint32) << 23).view(np.float32)
    t2 = (1.0 + np.float32(EXPQ_BETA) * r).astype(np.float32)
    return (t2 * t2 * e1).astype(np.float32)


def _expq_uop() -> UopConfig:
    u = UopConfig()
    u.enable_input(InpSel.SRC_0, 1)     # d0: x
    u.enable_input(InpSel.CONST_0, 2)   # d1: MAGIC
    u.enable_input(InpSel.CONST_1, 3)   # d2: BETA
    u.enable_input(InpSel.CONST_2, 4)   # d3: shift count (raw 23)
    u.enable_input(InpSel.ONE_F32, 5)   # d4: 1.0
    u.require_inp0 = ENABLE
    u.trigger = (Trigger.SRC_TENSOR_DONE, Trigger.NONE, Trigger.NONE)
    D = u.datapath_config
    D[0].enable_alu(AluOp.ADD, AluInp.PREV_DELAY_0, AluInp.PREV_DELAY_1)
    D[0].pass_through_delay(0, 1, 2, 3, 4)
    D[1].enable_alu(AluOp.SUBTRACT, AluInp.PREV_ALU_OUT, AluInp.PREV_DELAY_1)
    D[1].pass_through_delay(0, 2, 3, 4)
    D[1].enable_delay_from_src(DelayInp.PREV_ALU_OUT, 1)  # z
    D[2].enable_alu(AluOp.SUBTRACT, AluInp.PREV_DELAY_0, AluInp.PREV_ALU_OUT)
    D[2].pass_through_delay(1, 2, 3, 4)
    D[3].enable_alu(
        AluOp.LOGICAL_SHIFT_LEFT, AluInp.PREV_DELAY_1, AluInp.PREV_DELAY_3)
    D[3].pass_through_delay(2, 4)
    D[3].enable_delay_from_src(DelayInp.PREV_ALU_OUT, 0)  # r
    D[4].enable_alu(AluOp.MULTIPLY, AluInp.PREV_DELAY_0, AluInp.PREV_DELAY_2)
    D[4].pass_through_delay(4)
    D[4].enable_delay_from_src(DelayInp.PREV_ALU_OUT, 1)  # e1
    D[5].enable_alu(AluOp.ADD, AluInp.PREV_ALU_OUT, AluInp.PREV_DELAY_4)
    D[5].pass_through_delay(1)
    D[6].enable_alu(AluOp.MULTIPLY, AluInp.PREV_ALU_OUT, AluInp.PREV_ALU_OUT)
    D[6].pass_through_delay(1)
    D[7].enable_alu(AluOp.MULTIPLY, AluInp.PREV_ALU_OUT, AluInp.PREV_DELAY_1)
    u.enable_output(OutSel.ALU_OUT, OutPath.WR0_LO)
    return u


class _ExpqOp(DveOp):
    def __init__(self):
        object.__setattr__(self, "name", "EXPQ_ANT")
        object.__setattr__(
            self, "spec", Spec(body=Src0, reference=lambda *a: _expq_ref(a[0])))
        object.__setattr__(self, "subdim", False)
        object.__setattr__(self, "uops_sha", {})
        object.__setattr__(self, "perf_en", {})

    def compile(self, ver):
        return DveOpSpec(
            name=self.name,
            opcode=dve_ops.get_dve_sub_opcode(self.name),
            uops=[_expq_uop()],
            rd1_en=False,
        )


def _expq_register():
    if "EXPQ_ANT" in dve_ops._SUB_OPCODE_FOR_NAME:
        return next(op for op in dve_ops.OPS if op.name == "EXPQ_ANT")
    op = _ExpqOp()
    dve_ops.OPS.append(op)
    row = dve_ops._CUSTOM_DVE_ROW_BASE + len(dve_ops.OPS) - 1
    assert row < 0x20, "custom-DVE row overflow"
    dve_ops._SUB_OPCODE_FOR_NAME[op.name] = row
    dve_ops.CUSTOM_DVE_SPECS[op.name] = op.spec
    return op


def _expq_emit(nc, out_ap, in_ap, bias=None):
    op = _expq_register()
    return nc.vector._custom_dve(
        op, out=out_ap, in0=in_ap,
        s0=EXPQ_MAGIC if bias is None else bias,
        s1=EXPQ_BETA, imm2=EXPQ_SHIFT23)


_cache: dict = {}
last_results = None  # test.py introspection


def _build(mjt: int, reps: int = 1, debug: bool = False, exp_mode: str = 'dve', do_pvl: bool = True, do_phase1: bool = True, pvl_mode: str = 'both', ebufs: int = 24):
    """Build + compile the per-core Bass program for mjt j-tiles of 128."""
    mp = mjt * 128
    nc = bacc.Bacc("TRN2", target_bir_lowering=False, debug=False)

    d_xT = nc.declare_dram_parameter("xT", [2, 128, N_Q], F16, isOutput=False)
    d_cT = nc.declare_dram_parameter("cT", [2, 128, mp], F16, isOutput=False)
    d_wqk = nc.declare_dram_parameter("wqk", [128, 4 * HD], F16, isOutput=False)
    d_wv = nc.declare_dram_parameter("wv", [128, 2 * HD], F16, isOutput=False)
    d_wo = nc.declare_dram_parameter("wo", [128, DIM], F16, isOutput=False)
    d_bq = nc.declare_dram_parameter("bq", [128, 1], F32, isOutput=False)
    d_ef = nc.declare_dram_parameter("expf", [128, mjt], F32, isOutput=False)
    d_efw = nc.declare_dram_parameter(
        "expfw", [128, 32 * mjt], BF16, isOutput=False)
    d_out = nc.declare_dram_parameter("outT", [2, 128, N_Q], F16, isOutput=True)
    if debug:
        d_dbg = {
            nm: nc.declare_dram_parameter(f"dbg_{nm}", [128, width], F32,
                                          isOutput=True)
            for nm, width in [("l", N_Q), ("pv", N_Q), ("q", N_Q), ("k", mp),
                              ("v", mp), ("at", N_Q)]
        }

    with tile.TileContext(nc) as tc:
        with (
            tc.tile_pool(name="pin", bufs=1) as pin,
            tc.tile_pool(name="pwork", bufs=1) as pwork,
            tc.tile_pool(name="pe", bufs=ebufs) as pe_pool,
            tc.tile_pool(name="ps_s", bufs=4, space="PSUM") as ps_s,
            tc.tile_pool(name="ps_acc", bufs=1, space="PSUM") as ps_acc,
        ):
          for _rep in range(reps):
            # ---- loads ----
            xT_f = pin.tile([128, 2 * N_Q], F16)
            wqk_f = pin.tile([128, 4 * HD], F16)
            wv_f = pin.tile([128, 2 * HD], F16)
            bq_t = pin.tile([128, 1], F32)
            wq = [wqk_f[:, i * HD:(i + 1) * HD] for i in range(2)]
            wk = [wqk_f[:, (2 + i) * HD:(3 + i) * HD] for i in range(2)]
            wv = [wv_f[:, i * HD:(i + 1) * HD] for i in range(2)]
            cT_f = pin.tile([128, 2 * mp], F16)
            cT = [cT_f[:, i * mp:(i + 1) * mp] for i in range(2)]
            mh = (mp + 1) // 2
            ef = pin.tile([128, mjt], F32)
            efw = pin.tile([128, 32 * mjt], BF16)
            nc.sync.dma_start(wqk_f[:], d_wqk[:])
            nc.sync.dma_start(xT_f[:, 0:N_Q], d_xT[0])
            for ct in range(2):
                nc.sync.dma_start(cT_f[:, ct * mp:ct * mp + 256],
                                  d_cT[ct][:, 0:256])
            nc.sync.dma_start(bq_t[:], d_bq[:])
            nc.sync.dma_start(ef[:], d_ef[:])
            for ct in range(2):
                nc.sync.dma_start(
                    cT_f[:, ct * mp + 256:ct * mp + mh], d_cT[ct][:, 256:mh])
            nc.gpsimd.dma_start(xT_f[:, N_Q:2 * N_Q], d_xT[1])
            nc.gpsimd.dma_start(wv_f[:], d_wv[:])
            wo = pin.tile([128, DIM], F16)
            nc.gpsimd.dma_start(wo[:], d_wo[:])
            nc.gpsimd.dma_start(efw[:], d_efw[:])
            for ct in range(2):
                nc.gpsimd.dma_start(cT_f[:, ct * mp + mh:(ct + 1) * mp],
                                    d_cT[ct][:, mh:mp])

            # ---- persistent SBUF working tensors ----
            qT_hi = pwork.tile([128, N_Q], F16)   # [head*dim, i]
            kT_hi = pwork.tile([128, mp], F16)    # [head*dim, j] (log2e-scaled)
            vnat = pwork.tile([128, mp], BF16)    # [j_local, jt*128+head*dim]
            attnT = pwork.tile([128, N_Q], F16)
            linv = pwork.tile([128, N_Q], F32)
            outT = [pwork.tile([128, N_Q], F16, tag=f"outT{i}", name=f"outT{i}")
                    for i in range(2)]

            ones32 = pin.tile([128, 32], BF16)
            nc.vector.memset(ones32[:], 1.0)

            # warm the ACT exp table during the DMA phase
            warm = pwork.tile([128, 1], F32, tag="warm")
            nc.vector.memset(warm[:], 0.0)
            warm2 = pwork.tile([128, 1], F32, tag="warm2")
            nc.scalar.activation(warm2[:], warm[:], AF.Exp)

            # ---- PSUM accumulators ----
            pv_acc = ps_acc.tile([128, N_Q], F32, tag="pv")
            l_acc = ps_acc.tile([128, N_Q], F32, tag="l")
            nc.vector.memset(pv_acc[:], 0.0)
            nc.vector.memset(l_acc[:], 0.0)

            # ---- Q^T projection via pool tiles, evict with +bq ----
            for ih in range(2):
                sl = slice(ih * 512, ih * 512 + 512)
                qps = ps_s.tile([128, 512], F32, tag="s")
                for ct in range(2):
                    x0 = ih * N_Q + ct * 512
                    nc.tensor.matmul(
                        qps[:], wq[ct][:], xT_f[:, x0:x0 + 512],
                        start=(ct == 0), stop=(ct == 1),
                    )
                nc.vector.tensor_scalar_add(qT_hi[:, sl], qps[:], bq_t[:])

            # ---- phase 1: K/V projections for all j-tiles ----
            # K stages in spair banks 0/2 (jt parity), V in banks 1/3.
            def emit_kv(jt):
                j0 = jt * 128
                kps = ps_s.tile([128, 512], F32, tag="s", name="kps")
                for ct in range(2):
                    nc.tensor.matmul(
                        kps[:, 0:128], wk[ct][:], cT[ct][:, j0:j0 + 128],
                        start=(ct == 0), stop=(ct == 1),
                    )
                nc.scalar.copy(kT_hi[:, j0:j0 + 128], kps[:, 0:128])
                vps = ps_s.tile([128, 512], F32, tag="s", name="vps")
                for ct in range(2):
                    nc.tensor.matmul(
                        vps[:, 0:128], cT[ct][:, j0:j0 + 128], wv[ct][:],
                        start=(ct == 0), stop=(ct == 1),
                    )
                nc.vector.tensor_scalar_mul(
                    vnat[:, j0:j0 + 128], vps[:, 0:128], ef[:, jt:jt + 1])

            for jt in range(min(2, mjt)):
                emit_kv(jt)

            # ---- phase 2: half-rounds (head, i-half), depth-4 pipeline ----
            # S regions: 4 x [128,512] psum banks, cycled by round index.
            # Each round: 1 S matmul -> exp([128,512]) -> e tile; PV/L of the
            # previous tile's matching round interleave on the PE.
            def emit_pvl_hc(jt_p, h, c, e_t, last):
                j0p = jt_p * 128
                isl = slice(c * 512, c * 512 + 512)
                fin = last
                nc.tensor.matmul(
                    pv_acc[32 * h:32 * h + 32, isl],
                    vnat[:, j0p + 32 * h:j0p + 32 * h + 32], e_t[:],
                    start=False, stop=fin,
                    tile_position=(0, 32 * h),
                    skip_group_check=True,
                )
                nc.tensor.matmul(
                    l_acc[32 * h:32 * h + 32, isl],
                    efw[:, 32 * jt_p:32 * jt_p + 32], e_t[:],
                    start=False, stop=fin,
                    tile_position=(0, 32 * h),
                    skip_group_check=True,
                )

            # Two-tile interleaved rounds: alternating tiles doubles the
            # number of in-flight S->exp rounds, hiding the PE->DVE
            # drain+semaphore latency behind the other tile's work.
            prev_group = None  # (tiles, {(ti,h,c): e_tile})
            for base in range(0, mjt, 2):
                tiles = [t for t in (base, base + 1) if t < mjt]
                e_map = {}
                r = 0
                for h in range(HPC):
                    hp = slice(32 * h, 32 * h + 32)
                    for c in range(2):
                        for ti in tiles:
                            j0 = ti * 128
                            reg_t = ps_s.tile([128, 512], F32, tag="s",
                                              name="sreg")
                            reg = reg_t[:]
                            nc.tensor.matmul(
                                reg, kT_hi[hp, j0:j0 + 128],
                                qT_hi[hp, c * 512:c * 512 + 512],
                                start=True, stop=True,
                                tile_position=(32 * h, 0),
                            )
                            e_t = pe_pool.tile([128, 512], BF16, tag="e")
                            use_dve = (exp_mode == "dve") or (
                                exp_mode == "split" and c == 1)
                            if use_dve:
                                _expq_emit(nc, e_t[:], reg)
                            else:
                                nc.scalar.activation(e_t[:], reg, AF.Exp,
                                                     scale=LN2)
                            e_map[(ti, h, c)] = e_t
                        if prev_group is not None:
                            for tp in prev_group[0]:
                                emit_pvl_hc(tp, h, c,
                                            prev_group[1][(tp, h, c)],
                                            last=False)
                        r += 1
                        if r == 4:
                            for tn in tiles:
                                if tn + 2 < mjt:
                                    emit_kv(tn + 2)
                prev_group = (tiles, e_map)
            for tp in prev_group[0]:
                for h in range(HPC):
                    for c in range(2):
                        emit_pvl_hc(tp, h, c, prev_group[1][(tp, h, c)],
                                    last=(tp == prev_group[0][-1]
                                          and h == HPC - 1 and c == 1))

            # ---- debug dumps ----
            if debug:
                for nm, src, w in [("l", l_acc, N_Q), ("pv", pv_acc, N_Q),
                                   ("q", qT_hi, N_Q), ("k", kT_hi, mp),
                                   ("v", vnat, mp)]:
                    dt = pwork.tile([128, w], F32, tag=f"dbg_{nm}")
                    nc.vector.tensor_copy(dt[:], src[:])
                    nc.sync.dma_start(d_dbg[nm][:], dt[:])

            # ---- normalize + output projection ----
            for ih in range(2):
                sl = slice(ih * 512, ih * 512 + 512)
                nc.vector.reciprocal_approx_fast(linv[:, sl], l_acc[:, sl])
                nc.vector.tensor_tensor(
                    attnT[:, sl], pv_acc[:, sl], linv[:, sl],
                    mybir.AluOpType.mult)
                for dt in range(2):
                    ops_ = ps_s.tile([128, 512], F32, tag="s")
                    nc.tensor.matmul(
                        ops_[:], wo[:, dt * 128:dt * 128 + 128],
                        attnT[:, sl],
                        start=True, stop=True,
                    )
                    nc.scalar.copy(outT[dt][:, sl], ops_[:])
                    nc.sync.dma_start(d_out[dt][:, sl], outT[dt][:, sl])
            if debug:
                dt2 = pwork.tile([128, N_Q], F32, tag="dbg_at")
                nc.vector.tensor_copy(dt2[:], attnT[:])
                nc.sync.dma_start(d_dbg["at"][:], dt2[:])

    nc.compile()
    return nc


def build_in_maps(inputs, keeps, mjt):
    x = np.ascontiguousarray(np.asarray(inputs["x"], dtype=np.float32))
    context = np.ascontiguousarray(
        np.asarray(inputs["context"], dtype=np.float32))
    frag_mask = np.asarray(inputs["frag_mask"], dtype=np.float32)
    W_qkv = np.ascontiguousarray(np.asarray(inputs["W_qkv"], dtype=np.float32))
    b_qkv = np.asarray(inputs["b_qkv"], dtype=np.float32)
    W_out = np.ascontiguousarray(np.asarray(inputs["W_out"], dtype=np.float32))
    mp = mjt * 128
    in_maps = []
    for core in range(8):
        b, hh = core % B, core // B
        keep = keeps[b]
        cnt = len(keep)
        cT = np.zeros((DIM, mp), dtype=np.float32)
        cT[:, :cnt] = context[b][keep].T
        expf = np.zeros((mp,), dtype=np.float32)
        expf[:cnt] = np.exp(frag_mask[b][keep])
        hs = slice(hh * HD, (hh + 1) * HD)
        wq2 = W_qkv[:, hs].reshape(2, 128, HD)
        wk2 = (W_qkv[:, 256:512][:, hs] * np.float32(LOG2E)).reshape(2, 128, HD)
        wv2 = W_qkv[:, 512:768][:, hs].reshape(2, 128, HD)
        xr = x[b].T.reshape(2, 128, N_Q)
        efw = np.repeat(expf.reshape(mjt, 128).T[:, :, None], 32,
                        axis=2).reshape(128, mjt * 32)
        xih = np.stack([
            np.concatenate([xr[0][:, 0:512], xr[1][:, 0:512]], axis=1),
            np.concatenate([xr[0][:, 512:1024], xr[1][:, 512:1024]], axis=1),
        ])
        in_maps.append({
            "xT": np.ascontiguousarray(xih).astype(np.float16),
            "cT": np.ascontiguousarray(cT.reshape(2, 128, mp)).astype(
                np.float16),
            "wqk": np.ascontiguousarray(
                np.concatenate([wq2[0], wq2[1], wk2[0], wk2[1]], axis=1)
            ).astype(np.float16),
            "wv": np.ascontiguousarray(
                np.concatenate([wv2[0], wv2[1]], axis=1)).astype(np.float16),
            "wo": np.ascontiguousarray(W_out[hs, :]).astype(np.float16),
            "bq": np.ascontiguousarray(b_qkv[0:256][hs].reshape(128, 1)),
            "expf": np.ascontiguousarray(expf.reshape(mjt, 128).T),
            "expfw": np.ascontiguousarray(efw).astype(ml_dtypes.bfloat16),
        })
    return in_maps


def kernel(x, context, mask, frag_mask, W_qkv, b_qkv, W_out, b_out):
    global last_results
    mask = np.asarray(mask).astype(bool)
    b_out = np.asarray(b_out, dtype=np.float32)

    keeps = [np.nonzero(mask[b])[0] for b in range(B)]
    mjt = max(1, max((len(k) + 127) // 128 for k in keeps))

    key = (mjt, 1, False)
    if key not in _cache:
        _cache[key] = _build(mjt)
    nc = _cache[key]

    inputs = {"x": x, "context": context, "frag_mask": frag_mask,
              "W_qkv": W_qkv, "b_qkv": b_qkv, "W_out": W_out}
    in_maps = build_in_maps(inputs, keeps, mjt)

    res = run_bass_kernel_spmd(nc, in_maps, list(range(8)))
    last_results = res

    out = np.zeros((B, N_Q, DIM), dtype=np.float32)
    for core in range(8):
        b = core % B
        partial = res.results[core]["outT"].astype(np.float32).reshape(DIM, N_Q)
        out[b] += partial.T
    b_qkv = np.asarray(b_qkv, dtype=np.float32)
    out += (b_out + b_qkv[512:768] @ np.asarray(W_out, dtype=np.float32))[
        None, None, :]
    return out
